# revision 8
# baseline (speedup 1.0000x reference)
"""NNConv (gnn_message_passing) Bass kernel for 8 Trainium2 NeuronCores.

Strategy (edge-parallel, dst-sharded):
- Host relabels nodes with a permutation so that the 16384 nodes form 128
  "windows" of 128 nodes, each window receiving exactly U edges (by
  destination).  Core c owns windows [16c, 16c+16): 2048 nodes / 8192 edges.
  Pure SPMD; all per-core variation lives in the input data.
- Per 128-edge tile the work is split across four engines:
    P    = attr_aug^T @ Aperm          (PE matmul -> PSUM f32, bf16 inputs;
                                        A columns permuted to (o-block, i))
    q    = relu(P) * x[src]            (routed per chunk: Pool-engine fused
                                        scalar_tensor_tensor, DVE fused, or
                                        Act-engine relu + bf16 multiply)
    msum = sum_i q[:, o, i]            (DVE grouped tensor_reduce -> [128,64])
    agg += onehot(dst)^T @ msum        (single 64-row PE matmul per tile)
  The root term (x @ root) opens the same PSUM accumulation; bias is added
  during the PSUM->SBUF copy.
- Layer-1 x[src] is gathered on the host (dense DMA); layer-2 h1[src] uses
  an indirect DMA from the AllGathered h1 (bf16).
"""

import numpy as np
import ml_dtypes
from contextlib import ExitStack

import concourse.bass as bass
import concourse.tile as tile
from concourse import bacc, mybir
from concourse.bass import IndirectOffsetOnAxis
from concourse.bass_utils import run_bass_kernel_spmd

dt = mybir.dt
BF16 = ml_dtypes.bfloat16

N = 16384
E = 65536
NCORES = 8
P = 128                 # partitions / edges per tile
WINDOWS = 128           # global 128-node windows
WPC = WINDOWS // NCORES  # 16 windows per core
NPC = N // NCORES        # 2048 nodes per core
COUT = 64
CIN1 = 8
CIN2 = 64
OCH = 8                  # o-values per layer-2 chunk (chunk = OCH*CIN2 cols)
NCH2 = COUT // OCH       # 8 chunks in layer 2

# per-chunk relu-mult route for the 8 layer-2 chunks (GPSIMD/Pool cannot
# access PSUM, so PSUM is evacuated only by DVE-fused or Act-relu):
#   'df' = DVE fused (vector scalar_tensor_tensor from PSUM)
#   'ap' = Act relu (PSUM->SBUF bf16) + Pool tensor_tensor multiply
#   'ad' = Act relu + DVE tensor_tensor multiply
ROUTES2 = ('df', 'df', 'ap', 'ap', 'ap', 'ap', 'ap', 'ap')

_cached = {}


def _build_program(U):
    """Build the SPMD Bass program. U = edges per window (multiple of 128)."""
    T = U // P  # tiles per window
    EPC = WPC * U  # edges per core

    nc = bacc.Bacc("TRN2", target_bir_lowering=False, debug=False,
                   num_devices=NCORES)

    attrT_d = nc.dram_tensor("attrT", [3, EPC], dt.bfloat16, kind="ExternalInput").ap()
    src_d = nc.dram_tensor("srcidx", [EPC, 1], dt.int32, kind="ExternalInput").ap()
    dstrel_d = nc.dram_tensor("dstrel", [EPC, 1], dt.float32, kind="ExternalInput").ap()
    xg1_d = nc.dram_tensor("xg1", [EPC, CIN1], dt.bfloat16, kind="ExternalInput").ap()
    A1p_d = nc.dram_tensor("A1perm", [3, CIN1 * COUT], dt.bfloat16, kind="ExternalInput").ap()
    A2p_d = nc.dram_tensor("A2perm", [3, CIN2 * COUT], dt.bfloat16, kind="ExternalInput").ap()
    xT_d = nc.dram_tensor("xT", [CIN1, NPC], dt.bfloat16, kind="ExternalInput").ap()
    r1_d = nc.dram_tensor("root1", [CIN1, COUT], dt.bfloat16, kind="ExternalInput").ap()
    r2_d = nc.dram_tensor("root2", [CIN2, COUT], dt.bfloat16, kind="ExternalInput").ap()
    b1_d = nc.dram_tensor("bias1", [P, COUT], dt.float32, kind="ExternalInput").ap()
    b2_d = nc.dram_tensor("bias2", [P, COUT], dt.float32, kind="ExternalInput").ap()
    iota_d = nc.dram_tensor("iota", [P, P], dt.bfloat16, kind="ExternalInput").ap()
    out_d = nc.dram_tensor("out", [NPC, COUT], dt.float32, kind="ExternalOutput").ap()

    with tile.TileContext(nc) as tc, ExitStack() as ctx, \
            nc.allow_low_precision("bf16 msum; abs tolerance 2e-2"):
        consts = ctx.enter_context(tc.tile_pool(name="consts", bufs=1))
        meta = ctx.enter_context(tc.tile_pool(name="meta", bufs=4))
        xgp = ctx.enter_context(tc.tile_pool(name="xgp", bufs=4))
        ohp = ctx.enter_context(tc.tile_pool(name="ohp", bufs=4))
        qp = ctx.enter_context(tc.tile_pool(name="qp", bufs=6))
        rp = ctx.enter_context(tc.tile_pool(name="rp", bufs=6))
        msp = ctx.enter_context(tc.tile_pool(name="msp", bufs=4))
        rootp = ctx.enter_context(tc.tile_pool(name="rootp", bufs=2))
        outp = ctx.enter_context(tc.tile_pool(name="outp", bufs=3))
        pp = ctx.enter_context(tc.tile_pool(name="pp", bufs=4, space="PSUM"))
        aggp = ctx.enter_context(tc.tile_pool(name="aggp", bufs=3, space="PSUM"))
        dramp = ctx.enter_context(tc.tile_pool(name="dram", bufs=1, space="DRAM"))

        A1_s = consts.tile([3, CIN1 * COUT], dt.bfloat16)
        nc.sync.dma_start(A1_s[:], A1p_d[:])
        A2_s = consts.tile([3, CIN2 * COUT], dt.bfloat16)
        nc.sync.dma_start(A2_s[:], A2p_d[:])
        iota_s = consts.tile([P, P], dt.bfloat16)
        nc.sync.dma_start(iota_s[:], iota_d[:])
        r1_s = consts.tile([CIN1, COUT], dt.bfloat16)
        nc.sync.dma_start(r1_s[:], r1_d[:])
        r2_s = consts.tile([CIN2, COUT], dt.bfloat16)
        nc.sync.dma_start(r2_s[:], r2_d[:])
        b1_s = consts.tile([P, COUT], dt.float32)
        nc.sync.dma_start(b1_s[:], b1_d[:])
        b2_s = consts.tile([P, COUT], dt.float32)
        nc.sync.dma_start(b2_s[:], b2_d[:])

        # h1 slice (local) and allgathered h1 (global), bf16
        hloc = dramp.tile([NPC, COUT], dt.bfloat16)
        hglob = dramp.tile([N, COUT], dt.bfloat16)

        relu = mybir.ActivationFunctionType.Relu
        Alu = mybir.AluOpType

        def layer(cin, A_s, is_l1):
            # chunk geometry: layer2 chunk = OCH o-values x 64 i (512 cols);
            # layer1 single chunk = 64 o x 8 i (512 cols)
            if is_l1:
                nch, och, ich = 1, COUT, CIN1
            else:
                nch, och, ich = NCH2, OCH, CIN2
            cols = och * ich
            for w in range(WPC):
                aggw = aggp.tile([P, COUT], dt.float32, tag="aggw")
                if is_l1:
                    lhsTw = rootp.tile([CIN1, P], dt.bfloat16, tag="rootl1")
                    nc.sync.dma_start(lhsTw[:], xT_d[:, w * P:(w + 1) * P])
                    nc.tensor.matmul(aggw[:], lhsT=lhsTw[:], rhs=r1_s[:],
                                     start=True, stop=False)
                else:
                    lhsTw = rootp.tile([CIN2, P], dt.bfloat16, tag="rootl2")
                    nc.sync.dma_start_transpose(
                        lhsTw[:], hloc[w * P:(w + 1) * P, :])
                    nc.tensor.matmul(aggw[:], lhsT=lhsTw[:], rhs=r2_s[:],
                                     start=True, stop=False)
                for t in range(T):
                    e0 = (w * T + t) * P
                    attr_t = meta.tile([3, P], dt.bfloat16, tag="attr")
                    nc.sync.dma_start(attr_t[:], attrT_d[:, e0:e0 + P])
                    dstt = meta.tile([P, 1], dt.float32, tag="dst")
                    nc.sync.dma_start(dstt[:], dstrel_d[e0:e0 + P, :])

                    xg = xgp.tile([P, cin], dt.bfloat16,
                                  tag="xg1" if is_l1 else "xg2")
                    if is_l1:
                        nc.sync.dma_start(xg[:], xg1_d[e0:e0 + P, :])
                    else:
                        srct = meta.tile([P, 1], dt.int32, tag="src")
                        nc.sync.dma_start(srct[:], src_d[e0:e0 + P, :])
                        nc.gpsimd.indirect_dma_start(
                            out=xg[:], out_offset=None, in_=hglob[:],
                            in_offset=IndirectOffsetOnAxis(ap=srct[:, :1], axis=0))

                    oh = ohp.tile([P, P], dt.bfloat16, tag="oh")
                    nc.vector.tensor_scalar(
                        out=oh[:], in0=iota_s[:], scalar1=dstt[:, :1],
                        scalar2=None, op0=Alu.is_equal)

                    msum = msp.tile([P, COUT], dt.bfloat16, tag="ms")
                    # x broadcast over the o axis: [P, och, ich]
                    xgb = xg[:].unsqueeze(1).broadcast_to([P, och, ich])
                    for c in range(nch):
                        route = 'ap' if is_l1 else ROUTES2[c]
                        ppc = pp.tile([P, cols], dt.float32, tag="ppc")
                        nc.tensor.matmul(
                            ppc[:], lhsT=attr_t[:],
                            rhs=A_s[:, c * cols:(c + 1) * cols],
                            start=True, stop=True)
                        p3 = ppc[:].rearrange("p (o i) -> p o i", o=och)
                        qc = qp.tile([P, cols], dt.bfloat16, tag="qc")
                        q3 = qc[:].rearrange("p (o i) -> p o i", o=och)
                        if route == 'df':
                            eng = nc.vector
                            eng.scalar_tensor_tensor(
                                out=q3, in0=p3, scalar=0.0, in1=xgb,
                                op0=Alu.max, op1=Alu.mult)
                        else:
                            rc = rp.tile([P, cols], dt.bfloat16, tag="rc")
                            nc.scalar.activation(out=rc[:], in_=ppc[:],
                                                 func=relu)
                            r3 = rc[:].rearrange("p (o i) -> p o i", o=och)
                            eng = nc.gpsimd if route == 'ap' else nc.vector
                            eng.tensor_tensor(out=q3, in0=r3, in1=xgb,
                                              op=Alu.mult)
                        # grouped i-sum -> msum columns for this o-block
                        nc.vector.tensor_reduce(
                            out=msum[:, c * och:(c + 1) * och], in_=q3,
                            axis=mybir.AxisListType.X, op=Alu.add)
                    # one small scatter matmul per tile: agg += oh^T @ msum
                    nc.tensor.matmul(aggw[:], lhsT=oh[:], rhs=msum[:],
                                     start=False, stop=(t == T - 1),
                                     skip_group_check=True)
                # finalize window: add bias, write out
                if is_l1:
                    hw_ = outp.tile([P, COUT], dt.bfloat16, tag="h1w")
                    nc.vector.tensor_tensor(out=hw_[:], in0=aggw[:], in1=b1_s[:],
                                            op=Alu.add)
                    nc.sync.dma_start(hloc[w * P:(w + 1) * P, :], hw_[:])
                else:
                    ow = outp.tile([P, COUT], dt.float32, tag="outw")
                    nc.vector.tensor_tensor(out=ow[:], in0=aggw[:], in1=b2_s[:],
                                            op=Alu.add)
                    nc.sync.dma_start(out_d[w * P:(w + 1) * P, :], ow[:])

        layer(CIN1, A1_s, True)
        nc.gpsimd.collective_compute(
            "AllGather", mybir.AluOpType.bypass,
            replica_groups=[list(range(NCORES))],
            ins=[hloc[:].opt()], outs=[hglob[:].opt()])
        layer(CIN2, A2_s, False)

    nc.compile()
    return nc


def _perm_cols(cin, och):
    """Column permutation: new col (c, o_local, i) <- old col i*COUT + o."""
    idx = np.arange(cin * COUT)
    cols = och * cin
    c = idx // cols
    rem = idx % cols
    ol = rem // cin
    i = rem % cin
    o = c * och + ol
    return i * COUT + o


def _pack(edge_index):
    """Relabel nodes into 128 windows of 128 nodes / exactly U edges each.

    Returns (perm, U, order) where perm[orig_node] = new node id and
    order = edge permutation grouping edges by destination window, padded.
    """
    dst = np.asarray(edge_index[1], dtype=np.int64)
    deg = np.bincount(dst, minlength=N).astype(np.int64)
    # LPT greedy: descending degree, least-loaded window with free slots
    nodes = np.argsort(-deg, kind="stable")
    loads = np.zeros(WINDOWS, dtype=np.int64)
    slots = np.zeros(WINDOWS, dtype=np.int64)
    wof = np.empty(N, dtype=np.int64)  # window of node
    for v in nodes:
        open_w = np.flatnonzero(slots < P)
        w = open_w[np.argmin(loads[open_w])]
        wof[v] = w
        loads[w] += deg[v]
        slots[w] += 1
    # repair toward exact target load by swapping nodes between windows
    target = E // WINDOWS
    if loads.max() > target:
        by_wd = {}  # (window, degree) -> list of nodes
        for v in range(N):
            by_wd.setdefault((wof[v], deg[v]), []).append(v)
        for _ in range(100000):
            over = int(np.argmax(loads))
            under = int(np.argmin(loads))
            if loads[over] <= target:
                break
            delta = min(loads[over] - target, target - loads[under])
            # find a swap pair with degree difference = d, largest d first
            done = False
            for d in range(int(delta), 0, -1):
                for da in range(int(deg.max()), d - 1, -1):
                    la = by_wd.get((over, da))
                    lb = by_wd.get((under, da - d))
                    if la and lb:
                        a, b = la.pop(), lb.pop()
                        wof[a], wof[b] = under, over
                        by_wd.setdefault((under, da), []).append(a)
                        by_wd.setdefault((over, da - d), []).append(b)
                        loads[over] -= d
                        loads[under] += d
                        done = True
                        break
                if done:
                    break
            if not done:
                break
    U = int(np.ceil(loads.max() / P) * P)
    # perm: nodes sorted by window -> new ids
    new_order = np.argsort(wof * N + np.arange(N), kind="stable")
    perm = np.empty(N, dtype=np.int64)
    perm[new_order] = np.arange(N)
    # edge order: group by destination window, pad each window to U
    ew = wof[dst]
    eorder = np.argsort(ew, kind="stable")
    counts = np.bincount(ew, minlength=WINDOWS)
    padded = np.full(WINDOWS * U, -1, dtype=np.int64)
    pos = 0
    for w in range(WINDOWS):
        c = int(counts[w])
        padded[w * U:w * U + c] = eorder[pos:pos + c]
        pos += c
    return perm, U, padded


def kernel(x, edge_index, edge_attr, A1, b1, A2, b2, root1, bias1, root2, bias2):
    x = np.asarray(x, dtype=np.float32)
    edge_index = np.asarray(edge_index)
    edge_attr = np.asarray(edge_attr, dtype=np.float32)

    perm, U, padded = _pack(edge_index)
    key = U
    if key not in _cached:
        _cached[key] = _build_program(U)
    nc = _cached[key]

    src = np.asarray(edge_index[0], dtype=np.int64)
    dst = np.asarray(edge_index[1], dtype=np.int64)
    valid = padded >= 0
    pe = np.where(valid, padded, 0)
    # per padded-edge data
    a01 = edge_attr[pe]                      # [W*U, 2]
    aug = valid.astype(np.float32)
    attrT_all = np.stack([a01[:, 0] * aug, a01[:, 1] * aug, aug]).astype(BF16)
    srcn_all = np.where(valid, perm[src[pe]], 0).astype(np.int32)
    dstn = perm[dst[pe]]
    wof_e = np.arange(WINDOWS).repeat(U)
    dstrel_all = np.where(valid, dstn - wof_e * P, 0).astype(np.float32)

    x_pi = np.empty_like(x)
    x_pi[perm] = x
    x_bf = x_pi.astype(BF16)
    xg1_all = x_bf[srcn_all]                 # [W*U, CIN1] host pre-gather (l1)

    A1aug = np.concatenate([A1, b1[None, :]], axis=0).astype(BF16)
    A2aug = np.concatenate([A2, b2[None, :]], axis=0).astype(BF16)
    A1perm = A1aug[:, _perm_cols(CIN1, COUT)].copy()
    A2perm = A2aug[:, _perm_cols(CIN2, OCH)].copy()
    iota_np = np.broadcast_to(np.arange(P, dtype=np.float32), (P, P)).astype(BF16)
    b1_bc = np.broadcast_to(bias1, (P, COUT)).astype(np.float32).copy()
    b2_bc = np.broadcast_to(bias2, (P, COUT)).astype(np.float32).copy()
    shared = {
        "A1perm": A1perm, "A2perm": A2perm,
        "root1": np.asarray(root1.astype(BF16)),
        "root2": np.asarray(root2.astype(BF16)),
        "bias1": b1_bc, "bias2": b2_bc,
        "iota": np.asarray(iota_np),
    }
    EPC = WPC * U
    in_maps = []
    for c in range(NCORES):
        s = c * EPC
        m = dict(shared)
        m["attrT"] = attrT_all[:, s:s + EPC].copy()
        m["srcidx"] = srcn_all[s:s + EPC].reshape(EPC, 1).copy()
        m["dstrel"] = dstrel_all[s:s + EPC].reshape(EPC, 1).copy()
        m["xg1"] = np.ascontiguousarray(xg1_all[s:s + EPC])
        m["xT"] = np.ascontiguousarray(x_bf[c * NPC:(c + 1) * NPC].T)
        in_maps.append(m)

    res = run_bass_kernel_spmd(nc, in_maps, list(range(NCORES)),
                               **kernel.run_kwargs)
    kernel.last_result = res
    out_pi = np.concatenate([res.results[c]["out"] for c in range(NCORES)], axis=0)
    return out_pi[perm]


kernel.run_kwargs = {}
kernel.last_result = None
